# revision 25
# baseline (speedup 1.0000x reference)
"""AssimilationLoss Trainium2 kernel.

Reference math (x: [B, N, D] f32):
    loss = mean_b || sum_i x[b,i,:] / max(||x[b,i,:]||, eps) ||^2 / N^2

Sharding: data-parallel over B across 8 NeuronCores (one batch element per
core).  Each core streams its [N, D] shard once from HBM (16 MiB -> memory
bound), computes partial_b = || sum_i x_i/||x_i|| ||^2 locally, and the host
averages the 8 scalars.

Per-core pipeline over [128, 512] row-tiles (raw Bacc, manual semaphores):
  DMA : hybrid plan -- first chunks via HWDGE (f32r storage, starts ~2.5us),
        bulk via SWDGE with f32->bf16 cast on the wire (halves SBUF writes);
        big chunks early, 1-tile chunks last for a short tail.
  ACT : activation(Square, accum_out)     -> ss[p] = sum_d x[p,d]^2   (3/8 of tiles)
  DVE : affine_mul_reduce (custom op)     -> ss[p]                    (5/8 of tiles)
  ACT : sqrt (batched over tile groups)   -> norm[p]
  DVE : reciprocal                        -> inv[p] = 1/||x_p||  (bf16 / f32r)
  PE  : matmul(lhsT=inv, rhs=x_tile)      -> s[1, D] += sum_p x[p,:]/||x_p||
        (the per-row scaling rides the matmul weights; PSUM accumulates)
Epilogue: ACT square+acc of s -> scalar, DMA out from the sync HWDGE ring.

Measured on silicon (fast-mode min over repeated runs): ~55.9-56.3 us vs a
~47 us HBM wire floor.  Runs land bimodally (~56 vs ~65 us); the slow mode
is an environmental SDMA-engine-15 straggler, not kernel-dependent.
Alternatives measured SLOWER this session: all-HWDGE f32 streaming (58.0),
singleton tail groups + all-DVE tail squares (56.7), an extra prefix chunk
on the qAct ring (65 deterministic), partition-strided prefix DMAs (60.3).
"""

import numpy as np

import concourse.bacc as bacc
import concourse.mybir as mybir
from concourse.bass_utils import run_bass_kernel_spmd

def _ensure_ntff_hook():
    """Provide antenv.axon_hooks (NTFF profiling glue) if the image lacks it."""
    try:
        from antenv.axon_hooks import get_axon_ntff_profile_hook  # noqa: F401

        return
    except ImportError:
        pass
    import contextlib
    import ctypes
    import sys
    import types

    so_path = "/opt/axon/libaxon_pjrt.so"
    mod = types.ModuleType("antenv.axon_hooks")
    _state = {"hook": None}
    mod.set_axon_ntff_profile_hook = lambda h: _state.__setitem__("hook", h)
    mod.get_axon_ntff_profile_hook = lambda: _state["hook"]
    try:
        lib = ctypes.CDLL(so_path)
        if hasattr(lib, "axon_start_nrt_profile"):
            lib.axon_start_nrt_profile.argtypes = [
                ctypes.POINTER(ctypes.c_int64),
                ctypes.c_size_t,
            ]
            lib.axon_start_nrt_profile.restype = ctypes.c_int64
            lib.axon_stop_nrt_profile.argtypes = [ctypes.c_char_p]
            lib.axon_stop_nrt_profile.restype = ctypes.c_int64

            @contextlib.contextmanager
            def _hook(output_dir, device_ids):
                import jax

                jax.devices()
                if device_ids:
                    ids = (ctypes.c_int64 * len(device_ids))(*device_ids)
                    rc = lib.axon_start_nrt_profile(ids, len(device_ids))
                else:
                    rc = lib.axon_start_nrt_profile(None, 0)
                if rc != 0:
                    raise RuntimeError(f"axon_start_nrt_profile rc={rc}")
                try:
                    yield
                finally:
                    n = lib.axon_stop_nrt_profile(str(output_dir).encode())
                    if n <= 0:
                        print(f"ntff profile: rc={n} (no files?)", file=sys.stderr)

            _state["hook"] = _hook
    except OSError:
        pass
    import antenv

    sys.modules["antenv.axon_hooks"] = mod
    antenv.axon_hooks = mod


_ensure_ntff_hook()

B, N, D = 8, 8192, 512
P = 128

F32 = mybir.dt.float32
F32R = mybir.dt.float32r
BF16 = mybir.dt.bfloat16


def _build_nc():
    nc = bacc.Bacc("TRN2", target_bir_lowering=False, debug=False)
    x_ext = nc.dram_tensor("x", [N, D], F32R, kind="ExternalInput")
    out_ext = nc.dram_tensor("out", [1, 1], F32, kind="ExternalOutput")
    _body_raw(nc, x_ext.ap(), out_ext.ap())
    nc.compile()
    return nc


DMA_PLAN = (
    [(3, "hs", 128), (3, "hs", 128)]
    + [(8, "sw", 128)] * 4
    + [(4, "sw", 128)] * 3
    + [(2, "sw", 128)] * 5
    + [(1, "sw", 128)] * 4
)

GROUP = 4


def _on_act(t):
    return t % 8 in (1, 4, 6)


def _body_raw(nc, x, out):
    assert sum(m * pc for m, _, pc in DMA_PLAN) == N

    dmas = []
    tiles = []
    r0 = 0
    for di, (m, kind, pc) in enumerate(DMA_PLAN):
        dt = BF16 if kind == "sw" else F32R
        ap = nc.alloc_sbuf_tensor(f"xt{di}", [pc, m, D], dt).ap()
        dmas.append((kind, ap, r0, m, pc))
        for i in range(m):
            tiles.append((di, i, ap, kind, pc))
        r0 += m * pc
    assert r0 == N
    NT = len(tiles)

    groups = []
    t = 0
    while t < NT:
        kind = tiles[t][3]
        if t == NT - 1:
            cap = 1
        elif t + GROUP > NT - 1:
            cap = NT - 1 - t
        else:
            cap = GROUP
        g = 1
        while g < cap and t + g < NT and tiles[t + g][3] == kind:
            g += 1
        groups.append((t, g, kind))
        t += g

    ss = nc.alloc_sbuf_tensor("ss", [P, NT], F32).ap()
    nrm = nc.alloc_sbuf_tensor("nrm", [P, NT], F32).ap()
    inv_r = nc.alloc_sbuf_tensor("inv_r", [P, NT], F32R).ap()
    inv_b = nc.alloc_sbuf_tensor("inv_b", [P, NT], BF16).ap()
    ss_b = nc.alloc_sbuf_tensor("ss_b", [P, 1], F32).ap()
    sq_a = nc.alloc_sbuf_tensor("sq_a", [P, D], F32).ap()
    sq_v = nc.alloc_sbuf_tensor("sq_v", [P, D], F32).ap()
    s_sq = nc.alloc_sbuf_tensor("s_sq", [1, D], F32).ap()
    partial = nc.alloc_sbuf_tensor("partial", [1, 1], F32).ap()

    import contextlib

    _stack = contextlib.ExitStack()
    with (
        _stack,
        nc.psum_tensor([1, D], F32) as s_acc,
        nc.semaphore("amr_sem") as amr_sem,
        nc.semaphore("ssq_sem") as ssq_sem,
        nc.semaphore("norm_sem") as norm_sem,
        nc.semaphore("inv_sem") as inv_sem,
        nc.semaphore("mm_sem") as mm_sem,
        nc.semaphore("fin_sem") as fin_sem,
        nc.semaphore("out_sem") as out_sem,
        nc.Block() as block,
    ):
        dma_sems = [
            _stack.enter_context(nc.semaphore(f"dma{i}"))
            for i in range(len(DMA_PLAN))
        ]

        def dma_src(di):
            kind, ap, r0, m, pc = dmas[di]
            return x[r0 : r0 + m * pc, :].rearrange("(p n) d -> p n d", p=pc)

        def issue(eng, want):
            for di, (kind, ap, r0, m, pc) in enumerate(dmas):
                if kind == want:
                    eng.dma_start(out=ap, in_=dma_src(di)).then_inc(
                        dma_sems[di], 16
                    )

        @block.sync
        def _(sync):
            issue(sync, "hs")
            sync.wait_ge(fin_sem, 1)
            sync.dma_start(out=out, in_=partial).then_inc(out_sem, 16)
            sync.wait_ge(out_sem, 16)

        @block.gpsimd
        def _(gpsimd):
            issue(gpsimd, "sw")

        @block.scalar
        def _(scalar):
            scalar.activation(
                out=sq_a[:1, :1],
                in_=s_sq[:1, :1],
                func=mybir.ActivationFunctionType.Square,
            )
            scalar.activation(
                out=sq_a[:1, :1],
                in_=s_sq[:1, :1],
                func=mybir.ActivationFunctionType.Sqrt,
            )

            last_dma_waited = [-1]

            def tile_wait(t):
                di = tiles[t][0]
                if di > last_dma_waited[0]:
                    scalar.wait_ge(dma_sems[di], 16)
                    last_dma_waited[0] = di

            def squares(gi):
                gt0, gsize, kind = groups[gi]
                for t in range(gt0, gt0 + gsize):
                    if t == NT - 1 and not _on_act(t):
                        tile_wait(t)
                        di, i, ap, kind, pc = tiles[t]
                        apf = ap.bitcast(F32) if kind != "sw" else ap
                        scalar.activation(
                            out=sq_a[:pc, : D // 2],
                            in_=apf[:, i, D // 2 :],
                            func=mybir.ActivationFunctionType.Square,
                            accum_out=ss_b[:pc, :],
                        ).then_inc(ssq_sem, 1)
                    elif _on_act(t):
                        tile_wait(t)
                        di, i, ap, kind, pc = tiles[t]
                        apf = ap.bitcast(F32) if kind != "sw" else ap
                        scalar.activation(
                            out=sq_a[:pc, :],
                            in_=apf[:, i, :],
                            func=mybir.ActivationFunctionType.Square,
                            accum_out=ss[:pc, t : t + 1],
                        ).then_inc(ssq_sem, 1)

            def sqrt(gi):
                gt0, gsize, kind = groups[gi]
                scalar.wait_ge(amr_sem, gi + 1)
                scalar.activation(
                    out=nrm[:, gt0 : gt0 + gsize],
                    in_=ss[:, gt0 : gt0 + gsize],
                    func=mybir.ActivationFunctionType.Sqrt,
                ).then_inc(norm_sem, 1)

            squares(0)
            for gi in range(1, len(groups)):
                squares(gi)
                sqrt(gi - 1)
            sqrt(len(groups) - 1)

            scalar.wait_ge(mm_sem, len(groups))
            scalar.activation(
                out=s_sq,
                in_=s_acc.ap(),
                func=mybir.ActivationFunctionType.Square,
                accum_out=partial,
            ).then_inc(fin_sem, 1)

        @block.vector
        def _(vector):
            n_act = 0
            last_dma_waited = [-1]

            def tile_wait(t):
                di = tiles[t][0]
                if di > last_dma_waited[0]:
                    vector.wait_ge(dma_sems[di], 16)
                    last_dma_waited[0] = di

            def amrs(gi):
                nonlocal n_act
                gt0, gsize, kind = groups[gi]
                need_ssq_wait = False
                for t in range(gt0, gt0 + gsize):
                    if t == NT - 1 and not _on_act(t):
                        tile_wait(t)
                        di, i, ap, kind, pc = tiles[t]
                        apf = ap.bitcast(F32) if kind != "sw" else ap
                        vector.affine_mul_reduce(
                            out=sq_v[:pc, : D // 2],
                            accum_out=ss[:pc, t : t + 1],
                            in0=apf[:, i, : D // 2],
                            in1=apf[:, i, : D // 2],
                            scale=1.0,
                            bias=0.0,
                        )
                        n_act += 1
                        vector.wait_ge(ssq_sem, n_act)
                        vector.tensor_add(
                            ss[:pc, t : t + 1], ss[:pc, t : t + 1], ss_b[:pc, :]
                        )
                        continue
                    if _on_act(t):
                        n_act += 1
                        need_ssq_wait = True
                        continue
                    tile_wait(t)
                    di, i, ap, kind, pc = tiles[t]
                    apf = ap.bitcast(F32) if kind != "sw" else ap
                    vector.affine_mul_reduce(
                        out=sq_v[:pc, :],
                        accum_out=ss[:pc, t : t + 1],
                        in0=apf[:, i, :],
                        in1=apf[:, i, :],
                        scale=1.0,
                        bias=0.0,
                    )
                if need_ssq_wait:
                    vector.wait_ge(ssq_sem, n_act)
                tile_wait(gt0 + gsize - 1)
                vector.engine_nop().then_inc(amr_sem, 1)

            def recip(gi):
                gt0, gsize, kind = groups[gi]
                inv = inv_b if kind == "sw" else inv_r
                vector.wait_ge(norm_sem, gi + 1)
                with nc.allow_low_precision(reason="matmul weight dtype"):
                    vector.reciprocal(
                        out=inv[:, gt0 : gt0 + gsize],
                        in_=nrm[:, gt0 : gt0 + gsize],
                    ).then_inc(inv_sem, 1)

            amrs(0)
            for gi in range(1, len(groups)):
                amrs(gi)
                recip(gi - 1)
            recip(len(groups) - 1)

        @block.tensor
        def _(tensor):
            mm = 0
            for gi, (gt0, gsize, kind) in enumerate(groups):
                inv = inv_b if kind == "sw" else inv_r
                tensor.wait_ge(inv_sem, gi + 1)
                for t in range(gt0, gt0 + gsize):
                    di, i, ap, kind2, pc = tiles[t]
                    instr = tensor.matmul(
                        s_acc.ap(),
                        inv[:pc, t : t + 1],
                        ap[:, i, :],
                        start=(mm == 0),
                        stop=(mm == NT - 1),
                    )
                    mm += 1
                    if t == gt0 + gsize - 1:
                        instr.then_inc(mm_sem, 1)


_NC_CACHE = {}


def _get_nc():
    if "nc" not in _NC_CACHE:
        _NC_CACHE["nc"] = _build_nc()
    return _NC_CACHE["nc"]


def kernel(x: np.ndarray, trace: bool = False):
    assert x.shape == (B, N, D), x.shape
    nc = _get_nc()
    in_maps = [{"x": np.ascontiguousarray(x[b], dtype=np.float32)} for b in range(B)]
    res = None
    for attempt in range(3):
        try:
            res = run_bass_kernel_spmd(
                nc, in_maps, core_ids=list(range(B)), trace=trace
            )
            break
        except Exception:
            if attempt == 2:
                raise
            import time

            time.sleep(25)
    partials = [float(r["out"][0, 0]) for r in res.results]
    val = np.float32(np.sum(np.asarray(partials, dtype=np.float64)) / (N * N) / B)
    if trace:
        return val, res
    return val
